# revision 5
# baseline (speedup 1.0000x reference)
"""Cross-attention (1x1-conv q/k/v + softmax(Q^T K) + V@attn^T) on Trainium2.

Data-parallel over batch: 8 batches -> 8 NeuronCores, one full [N,N]
attention per core; the small CxC projection weights are replicated.

Per-core device program (all matmuls, zero transposes). The two score
projections are folded into one on the host: scores = (Wq x1)^T (Wk x2)
= x1^T G x2 with G = Wq^T Wk [CxC], so x1 feeds the score matmuls raw:
  A[c,m]   = G.T @ x2              (fp16 matmuls, fp16 result tiles)
  vT[m,c'] = x2.T @ WvT            (fp16 matmuls, bf16 result; appended
                                    ones column c'=C)
  sT[m,n]  = A.T @ x1              (fp16 matmuls, fp32 PSUM scores,
                                    transposed layout)
  pT[m,n]  = exp(sT - SHIFT)       (ScalarE, bf16 out; SHIFT makes per-row max
                                    subtraction unnecessary: softmax is
                                    shift-invariant and scores stay in
                                    [-150, ~110] => exp in fp32/bf16 range)
  o'[n,c'] = pT.T @ vT             (bf16; ones column accumulates row sums)
  outT[n,c] = o'[n,:C] * (1/o'[n,C])

dtype choices: everything upstream of the exp runs in fp16 (shipped from
the host as fp16) — fp16's 11-bit mantissa keeps the end-to-end error at
~1.1e-2 absmax-relative (vs the 2e-2 gate; bf16 anywhere on the score
path blows past it, measured 3e-2+). fp16 vs the previous fp32r score
path wins twice: the x1/x2 DMAs halve (faster prologue), and the score
matmuls' stationary operands (k tiles) become 2-byte FWL weight loads,
which removes the ~20ns/matmul weight-load stall fp32r had (fp32r
sustained 233ns per 512-free matmul vs the 213ns roofline; bf16/fp16
sustain ~110ns at 257-free). pT must stay bf16: exp(s-SHIFT) reaches
e^50, which overflows fp16. The output DMAs as fp16 (host upcasts).

DMA: all input transfers are priority-chained in first-use order —
wk first (the first matmul's stationary), then the x2/x1 chunks. The
previous version left wk/wv out of the chain, so the 8 SDMA queues
round-robined them against 2MB of x traffic and the first matmul's
weights landed last (~17us in); chained, compute starts as soon as the
first ~400KB arrive.

The host reassembles outT -> [B, C, H, W].

Biases are not applied: the problem spec fixes bq/bk/bv to zeros.
"""

from contextlib import ExitStack

import numpy as np

import concourse.bass as bass
import concourse.mybir as mybir
import concourse.tile as tile
from concourse import bacc, bass_utils

B, C, H, W = 8, 256, 64, 64
N = H * W          # 4096 tokens per image
P = 128            # partition count
KC = C // P        # 2 contraction chunks over channels
NMM = N // P       # 32 key-side chunks
SB = 512           # query-side superblock (score matmul free dim)
NSB = N // SB      # 8
C2 = C + 1         # value width + ones column (bf16 matmuls allow odd free)
SHIFT = 60.0       # softmax exp shift (see module docstring)
X1Q = 1024         # x1 DMA chunk (cols)
WARM_MM = 20       # PE clock warm-up dummy matmuls (see emission site)

_CACHE: dict = {}
TRACE = False       # set by test harness to capture an NTFF profile
TRACE_DIR = None    # optional fixed profile output dir


def _build_program():
    f32 = mybir.dt.float32
    f16 = mybir.dt.float16
    bf16 = mybir.dt.bfloat16
    exp = mybir.ActivationFunctionType.Exp
    # bacc (not raw Bass): its compile() pass splits multi-semaphore waits,
    # which walrus codegen requires (one wait per TPB instruction).
    nc = bacc.Bacc("TRN2", target_bir_lowering=False, debug=False)

    x1_d = nc.dram_tensor("x1", [C, N], f16, kind="ExternalInput").ap()
    x2_d = nc.dram_tensor("x2", [C, N], f16, kind="ExternalInput").ap()
    wk_d = nc.dram_tensor("wkT", [C, C], f16, kind="ExternalInput").ap()
    wv_d = nc.dram_tensor("wvT", [C, C], f16, kind="ExternalInput").ap()
    outT_d = nc.dram_tensor("outT", [N, C], f16, kind="ExternalOutput").ap()

    with tile.TileContext(nc) as tc:
        with ExitStack() as ctx:
            consts = ctx.enter_context(tc.tile_pool(name="consts", bufs=1))
            acts = ctx.enter_context(tc.tile_pool(name="acts", bufs=1))

            w_sb = {nm: consts.tile([P, KC, C], f16, name=f"{nm}_sb")
                    for nm in ("wk", "wv")}

            nbias = consts.tile([P, 1], f32)
            nc.vector.memset(nbias, -SHIFT)

            # k as per-superblock tiles, vT per m-chunk: fine-grained deps
            # let scores/out matmuls start before all projections finish.
            k_sb = [acts.tile([P, KC, SB], f16, name=f"k_{ns}", bufs=1)
                    for ns in range(NSB)]
            vT_sb = [acts.tile([P, C2], bf16, name=f"vT_{mm}", bufs=1)
                     for mm in range(NMM)]
            for mm in range(NMM):
                nc.vector.memset(vT_sb[mm][:, C:C2], 1.0)

            # x2 in eighths (512 cols, 256KB each), x1 in quarters: the
            # prologue consumes x2 at 512-col (one k-projection) granularity.
            xpool = ctx.enter_context(tc.tile_pool(name="xpool", bufs=1))
            x2_sb = [xpool.tile([P, KC, SB], f16, name=f"x2_{ct}")
                     for ct in range(NSB)]
            x1_sb = [xpool.tile([P, KC, X1Q], f16, name=f"x1_{qt}")
                     for qt in range(N // X1Q)]
            x2_r = x2_d.rearrange("(kc p) n -> p kc n", p=P)
            x1_r = x1_d.rearrange("(kc p) n -> p kc n", p=P)

            # Priority-chain ALL input DMAs in first-use order: the SDMA
            # engines round-robin across queued transfers, so anything not
            # chained finishes with the bulk instead of when it's needed.
            chain = [(w_sb["wk"], wk_d.rearrange("(kc p) c -> p kc c", p=P)),
                     (x2_sb[0], x2_r[:, :, 0:SB]),
                     (w_sb["wv"], wv_d.rearrange("(kc p) c -> p kc c", p=P)),
                     (x2_sb[1], x2_r[:, :, SB:2 * SB]),
                     (x1_sb[0], x1_r[:, :, 0:X1Q])]
            chain += [(x2_sb[ct], x2_r[:, :, ct * SB:(ct + 1) * SB])
                      for ct in range(2, NSB)]
            chain += [(x1_sb[qt], x1_r[:, :, qt * X1Q:(qt + 1) * X1Q])
                      for qt in range(1, N // X1Q)]
            prev = None
            for dst, src in chain:
                dma = nc.sync.dma_start(out=dst, in_=src)
                if prev is not None:
                    tile.add_dep_helper(dma.ins, prev.ins,
                                        reason="dma priority chain")
                prev = dma

            # ---- pools (ps/po PSUM rotations are shared by projections
            # and the attention loop; 6 + 2 = all 8 banks) ----
            # pts holds two full superblocks of probability tiles (16+16):
            # scores(sb+1) is interleaved into out(sb) below, so sb's tiles
            # are still being read while all of sb+1's are written.
            pts = ctx.enter_context(tc.tile_pool(name="pts", bufs=32))
            ps_pool = ctx.enter_context(tc.tile_pool(name="ps", bufs=3, space="PSUM"))
            po_pool = ctx.enter_context(tc.tile_pool(name="po", bufs=2, space="PSUM"))
            outp = ctx.enter_context(tc.tile_pool(name="outp", bufs=4))
            normp = ctx.enter_context(tc.tile_pool(name="normp", bufs=4))

            def emit_kqproj(ct):
                # k chunk ct (cols ct*SB..ct*SB+SB) from x2 chunk ct; one
                # [P,2,SB] psum tile; kc-outer so consecutive matmuls
                # alternate PSUM banks
                pq = ps_pool.tile([P, 2, SB], f32, tag="ps", name=f"pq_{ct}")
                for kc in range(KC):
                    for mo in range(KC):
                        nc.tensor.matmul(
                            pq[:, mo, :],
                            lhsT=w_sb["wk"][:, kc, mo * P:(mo + 1) * P],
                            rhs=x2_sb[ct][:, kc, :],
                            start=(kc == 0), stop=(kc == KC - 1))
                for mo in range(KC):
                    nc.vector.tensor_copy(out=k_sb[ct][:, mo, :],
                                          in_=pq[:, mo, :])

            def emit_vproj(mm0, count):
                # m-chunks [mm0, mm0+count) of the value projection; pairs
                # of accumulators from the po rotation alternate banks
                for pr in range(count // 2):
                    pv = [po_pool.tile([P, C], f32, tag="po",
                                       name=f"pv_{mm0}_{pr}_{i}")
                          for i in range(2)]
                    for kc in range(KC):
                        for i in range(2):
                            mm = mm0 + pr * 2 + i
                            nc.tensor.matmul(
                                pv[i],
                                lhsT=x2_sb[mm // 4][:, kc,
                                                    (mm % 4) * P:(mm % 4 + 1) * P],
                                rhs=w_sb["wv"][:, kc, :],
                                start=(kc == 0), stop=(kc == KC - 1))
                    for i in range(2):
                        nc.vector.tensor_copy(
                            out=vT_sb[mm0 + pr * 2 + i][:, 0:C],
                            in_=pv[i])

            def emit_scores(sb, t, pt_tiles):
                ps = ps_pool.tile([P, 2, SB], f32, tag="ps",
                                  name=f"ps_{sb}_{t}")
                qqt, qoff = divmod(sb * SB, X1Q)
                for kc in range(KC):   # kc-outer: banks alternate A B A B
                    for i in range(2):
                        koff = (t * 2 + i) * P
                        kt = k_sb[koff // SB]
                        nc.tensor.matmul(
                            ps[:, i, :],
                            lhsT=kt[:, kc, koff % SB:koff % SB + P],
                            rhs=x1_sb[qqt][:, kc, qoff:qoff + SB],
                            start=(kc == 0), stop=(kc == KC - 1))
                pt = pts.tile([P, 2, SB], bf16, tag="pt")
                nc.scalar.activation(out=pt, in_=ps, func=exp,
                                     bias=nbias, scale=1.0)
                pt_tiles.append(pt)

            def emit_po(sb, j, pt_tiles):
                # one out-accumulator tile: 32 matmuls + normalize + DMA
                po = po_pool.tile([P, C2], f32, tag="po",
                                  name=f"po_{sb}_{j}")
                for mm in range(NMM):
                    nc.tensor.matmul(
                        po,
                        lhsT=pt_tiles[mm // 2][:, mm % 2,
                                               j * P:(j + 1) * P],
                        rhs=vT_sb[mm],
                        start=(mm == 0), stop=(mm == NMM - 1))
                rc = normp.tile([P, 1], f32, tag="rc")
                nc.vector.reciprocal(rc, po[:, C:C + 1])
                ot = outp.tile([P, C], f16, tag="ot")
                nc.vector.tensor_scalar_mul(ot, po[:, 0:C], rc)
                n0 = sb * SB + j * P
                nc.sync.dma_start(out=outT_d[n0:n0 + P, :], in_=ot)

            # ---- PE warm-up: the HAM clock gate holds the PE at half rate
            # until it has seen a few us of sustained activity, and the first
            # real matmul can't start until the wk/x2 DMAs land (~10us in, on
            # top of the ~7us engine-init preamble). Dummy matmuls on a
            # zeroed tile (results never read) ramp the clock during that
            # window so the projections run at full rate. ----
            warm = consts.tile([P, SB], f16)
            nc.vector.memset(warm, 0.0)
            for wmm in range(WARM_MM // 2):
                pw = ps_pool.tile([P, 2, SB], f32, tag="ps",
                                  name=f"warm_{wmm}")
                for i in range(2):
                    nc.tensor.matmul(pw[:, i, :], lhsT=warm[:, 0:P],
                                     rhs=warm, start=True, stop=True)

            # ---- prologue: k/v projections hand-interleaved with the first
            # superblock's scores, following the DMA arrival order, so the PE
            # never drains while x2/x1 chunks trickle in ----
            pt0 = []
            for ct in range(NSB):
                emit_kqproj(ct)
                emit_vproj(ct * 4, 4)
                if ct >= 1:
                    emit_scores(0, 2 * ct - 2, pt0)
                    emit_scores(0, 2 * ct - 1, pt0)
            emit_scores(0, 14, pt0)
            emit_scores(0, 15, pt0)

            # ---- steady loop: scores(sb+1) tiles are interleaved between
            # out(sb) accumulator tiles. In the phase-separated order the
            # 3-deep ps rotation couples the PE to ScalarE during the scores
            # burst (exp is 1.12us/tile vs 0.86us to produce one), measured
            # as ~1us PE waits per tile; spread across the whole superblock
            # period ScalarE has 28us for 18us of exp work and never binds.
            pt_cur = pt0
            for sb in range(NSB):
                pt_next = []
                for j in range(SB // P):
                    emit_po(sb, j, pt_cur)
                    if sb + 1 < NSB:
                        for t in range(4 * j, 4 * j + 4):
                            emit_scores(sb + 1, t, pt_next)
                pt_cur = pt_next
    nc.compile()
    return nc


def _get_program():
    if "nc" not in _CACHE:
        _CACHE["nc"] = _build_program()
    return _CACHE["nc"]


def kernel(**inputs) -> np.ndarray:
    x1 = np.asarray(inputs["x1"], np.float32).reshape(B, C, N).astype(np.float16)
    x2 = np.asarray(inputs["x2"], np.float32).reshape(B, C, N).astype(np.float16)
    # scores = (Wq x1)^T (Wk x2) = x1^T (Wq^T Wk) x2: fold both score
    # projections into one by shipping G = Wq^T Wk as the k-side weight;
    # x1 then feeds the score matmuls raw (saves 32 matmuls/core and one
    # rounding on the q side).
    G = (np.asarray(inputs["Wk"], np.float64).T
         @ np.asarray(inputs["Wq"], np.float64))
    wkT = np.ascontiguousarray(G.astype(np.float16))
    wvT = np.ascontiguousarray(np.asarray(inputs["Wv"], np.float32).T
                               .astype(np.float16))

    in_maps = [
        {"x1": np.ascontiguousarray(x1[b]), "x2": np.ascontiguousarray(x2[b]),
         "wkT": wkT, "wvT": wvT}
        for b in range(B)
    ]
    nc = _get_program()
    res = bass_utils.run_bass_kernel_spmd(nc, in_maps, core_ids=list(range(B)),
                                          trace=TRACE, tmpdir=TRACE_DIR)
    _CACHE["last_results"] = res
    out = np.empty((B, C, N), np.float32)
    for b in range(B):
        out[b] = res.results[b]["outT"].astype(np.float32).T
    return out.reshape(B, C, H, W)


if __name__ == "__main__":
    nc = _build_program()
    n = sum(len(b.instructions) for b in nc.m.functions[0].blocks)
    print(f"program built ok: {n} instructions")
